# revision 2
# baseline (speedup 1.0000x reference)
"""MoE (8 experts, top-2, shared expert) Trainium2 kernel — v3.

Expert-parallel over 8 NeuronCores, bf16 matmuls (f32 PSUM accum).
Host performs only the dispatch decision (top-2 ids -> compact per-expert
token lists + scatter/gather index tables) and data layout; all FP model
math — router logits/gates, expert SwiGLU, shared expert, combine adds —
runs on device.

Structure per core (SPMD, identical program, per-core data):
  A:  gates[c] = (router_e . x_c) for this expert's compact tokens
  B:  ht[I, C] = silu(w1 @ xg) * (w3 @ xg)          (compact tokens)
  C:  y[ct]    = gate * (ht.T @ w2t); indirect-scatter rows (f32) into
      sendbuf laid out [dest_core, slot, H] (pads -> trash rows)
  A2A: AllToAll(sendbuf) -> recvbuf  (only real expert rows move)
  S:  shared expert, data-parallel over tokens: this core's OWN 256-token
      slice through the FULL shared expert (I=1408; no partial partition
      tiles) — emitted after the A2A so PE compute hides the collective.
  T:  ysh[t] += recv[g1[t]]; ysh[t] += recv[g2[t]]  (indirect gather-ADD
      on the Pool queue; no post-collective work on Act/DVE/PE) -> out

v4: the sendbuf is built by dense ybuf writes + indirect GATHERS
(slot -> compact row) instead of indirect scatters: SWDGE scatters cost
~10us serial descriptor processing per 128-row tile and were delaying
the A2A launch past the shared-expert cover; gathers are ~0.7us.
"""

import numpy as np

H = 1024          # hidden
I = 1408          # moe intermediate
E = 8             # experts == cores
T = 2048          # tokens (2*1024)
TOPK = 2
C = 544           # compact per-expert token capacity (max observed 540)
P = 96            # per (expert, owner) pair capacity (max observed 78)
TSL = T // E      # 256: output token slice per core
KT = H // 128     # 8 contraction tiles over H
IT = I // 128     # 11 tiles over I
NCORES = 8

_BUILD_CACHE = {}
_PHASE_MARKS = []


def _build(reps=1, cap=None, pcap=None):
    import concourse.bacc as bacc
    import concourse.bass as bass
    import concourse.mybir as mybir
    from concourse import tile
    from contextlib import ExitStack

    f32 = mybir.dt.float32
    bf16 = mybir.dt.bfloat16
    i32 = mybir.dt.int32
    AF = mybir.ActivationFunctionType
    MUL = mybir.AluOpType.mult
    ADD = mybir.AluOpType.add

    C_ = cap or C
    P_ = pcap or P
    n_ch = max(1, (C_ + 511) // 512)
    CH_ = -(-C_ // n_ch)
    CH_ = -(-CH_ // 16) * 16          # B token-chunk length, 16-aligned
    PT = []                           # token partition tiles: 128s + rem
    left = C_
    while left > 0:
        PT.append(min(128, left))
        left -= 128
    CT_ = len(PT)
    SROWS = E * P_ + 128              # sendbuf rows incl. per-lane trash
    TT = TSL // 128                   # own-slice token tiles (2)

    nc = bacc.Bacc("TRN2", target_bir_lowering=False, debug=False,
                   num_devices=NCORES)

    xgb = nc.declare_dram_parameter("xgb", [128, KT * C_], bf16,
                                    isOutput=False)
    xsh = nc.declare_dram_parameter("xsh", [128, KT * TSL], bf16,
                                    isOutput=False)
    w1t = nc.declare_dram_parameter("w1t", [128, IT, KT * 128], bf16,
                                    isOutput=False)
    w3t = nc.declare_dram_parameter("w3t", [128, IT, KT * 128], bf16,
                                    isOutput=False)
    w2t = nc.declare_dram_parameter("w2t", [128, IT * H], bf16,
                                    isOutput=False)
    s1t = nc.declare_dram_parameter("s1t", [128, IT, KT * 128], bf16,
                                    isOutput=False)
    s3t = nc.declare_dram_parameter("s3t", [128, IT, KT * 128], bf16,
                                    isOutput=False)
    s2t = nc.declare_dram_parameter("s2t", [128, IT * H], bf16,
                                    isOutput=False)
    rwe = nc.declare_dram_parameter("rwe", [128, KT * 16], bf16,
                                    isOutput=False)
    SGT = (E * P_) // 128                 # sendbuf slot tiles (768/128=6)
    gsnd = nc.declare_dram_parameter("gsnd", [128, SGT], i32, isOutput=False)
    g1i = nc.declare_dram_parameter("g1i", [128, TT], i32, isOutput=False)
    g2i = nc.declare_dram_parameter("g2i", [128, TT], i32, isOutput=False)
    out = nc.declare_dram_parameter("out", [TSL, H], f32, isOutput=True)

    ybuf = nc.dram_tensor("ybuf", [CT_ * 128, H], f32)
    sendbuf = nc.dram_tensor("sendbuf", [E * P_, H], f32)
    recvbuf = nc.dram_tensor("recvbuf", [E * P_, H], f32)

    with tile.TileContext(nc) as tc, ExitStack() as ctx:
        sres = ctx.enter_context(tc.tile_pool(name="sres", bufs=1))
        sstr = ctx.enter_context(tc.tile_pool(name="sstr", bufs=2))
        work = ctx.enter_context(tc.tile_pool(name="work", bufs=2))
        psA = ctx.enter_context(tc.tile_pool(name="psA", bufs=2, space="PSUM"))
        psB = ctx.enter_context(tc.tile_pool(name="psB", bufs=2, space="PSUM"))
        psY = ctx.enter_context(tc.tile_pool(name="psY", bufs=4, space="PSUM"))

        for _rep in range(reps):
            # ---- resident loads (sync HWDGE queue, deadline order) ----
            rwe_sb = sres.tile([128, KT * 16], bf16, tag="rwe_sb",
                               name="rwe_sb")
            nc.sync.dma_start(rwe_sb[:], rwe[:, :])
            xgb_sb = sres.tile([128, KT * C_], bf16, tag="xgb_sb",
                               name="xgb_sb")
            w1_sb = sres.tile([128, IT * KT * 128], bf16, tag="w1_sb",
                              name="w1_sb")
            w3_sb = sres.tile([128, IT * KT * 128], bf16, tag="w3_sb",
                              name="w3_sb")
            # interleave the first chunks so B's k-loop starts with the load
            nc.sync.dma_start(xgb_sb[:, 0:C_], xgb[:, 0:C_])
            nc.sync.dma_start(w1_sb[:, 0:KT * 128], w1t[:, 0, :])
            nc.sync.dma_start(w3_sb[:, 0:KT * 128], w3t[:, 0, :])
            for k in range(1, KT):
                nc.sync.dma_start(xgb_sb[:, k * C_:(k + 1) * C_],
                                  xgb[:, k * C_:(k + 1) * C_])
            for i in range(1, IT):
                nc.sync.dma_start(
                    w1_sb[:, i * KT * 128:(i + 1) * KT * 128], w1t[:, i, :])
                nc.sync.dma_start(
                    w3_sb[:, i * KT * 128:(i + 1) * KT * 128], w3t[:, i, :])
            w2_sb = sres.tile([128, IT * H], bf16, tag="w2_sb", name="w2_sb")
            nc.sync.dma_start(w2_sb[:], w2t[:, :])
            xsh_sb = sres.tile([128, KT * TSL], bf16, tag="xsh_sb",
                               name="xsh_sb")
            nc.sync.dma_start(xsh_sb[:], xsh[:, :])
            s2_sb = sres.tile([128, IT * H], bf16, tag="s2_sb", name="s2_sb")
            nc.sync.dma_start(s2_sb[:], s2t[:, :])
            s1_sb = sres.tile([128, IT * KT * 128], bf16, tag="s1_sb",
                              name="s1_sb")
            s3_sb = sres.tile([128, IT * KT * 128], bf16, tag="s3_sb",
                              name="s3_sb")
            for i in range(IT):
                nc.sync.dma_start(
                    s1_sb[:, i * KT * 128:(i + 1) * KT * 128], s1t[:, i, :])
                nc.sync.dma_start(
                    s3_sb[:, i * KT * 128:(i + 1) * KT * 128], s3t[:, i, :])
            gsnd_sb = sres.tile([128, SGT], i32, tag="gsnd_sb",
                                name="gsnd_sb")
            nc.sync.dma_start(gsnd_sb[:], gsnd[:, :])
            g1i_sb = sres.tile([128, TT], i32, tag="g1i_sb", name="g1i_sb")
            nc.sync.dma_start(g1i_sb[:], g1i[:, :])
            g2i_sb = sres.tile([128, TT], i32, tag="g2i_sb", name="g2i_sb")
            nc.sync.dma_start(g2i_sb[:], g2i[:, :])

            _PHASE_MARKS.append(('loads', nc.next_id()))
            # ---- B: expert ht[I, C] = silu(w1@x) * (w3@x) ----
            ht_sb = sres.tile([128, IT * C_], bf16, tag="ht_sb", name="ht_sb")
            for i in range(IT):
                for cc in range(n_ch):
                    c0 = cc * CH_
                    cw = min(CH_, C_ - c0)
                    psa = psA.tile([128, CH_], f32, tag="a", name="psa",
                                   space="PSUM")
                    psb = psB.tile([128, CH_], f32, tag="b", name="psb",
                                   space="PSUM")
                    for k in range(KT):
                        nc.tensor.matmul(
                            psa[:, :cw],
                            lhsT=w1_sb[:, (i * KT + k) * 128:
                                       (i * KT + k + 1) * 128],
                            rhs=xgb_sb[:, k * C_ + c0: k * C_ + c0 + cw],
                            start=(k == 0), stop=(k == KT - 1))
                    for k in range(KT):
                        nc.tensor.matmul(
                            psb[:, :cw],
                            lhsT=w3_sb[:, (i * KT + k) * 128:
                                       (i * KT + k + 1) * 128],
                            rhs=xgb_sb[:, k * C_ + c0: k * C_ + c0 + cw],
                            start=(k == 0), stop=(k == KT - 1))
                    sact = work.tile([128, CH_], f32, tag="sact",
                                     name="sact")
                    nc.scalar.activation(sact[:, :cw], psa[:, :cw], AF.Silu)
                    nc.vector.tensor_tensor(
                        out=ht_sb[:, i * C_ + c0: i * C_ + c0 + cw],
                        in0=sact[:, :cw], in1=psb[:, :cw], op=MUL)

            # ---- A: gates[c] = router_e . x_c (raw logit) ----
            gates_sb = sres.tile([128, CT_], f32, tag="gates_sb",
                                 name="gates_sb")
            for ct in range(CT_):
                pt = PT[ct]
                psl = psY.tile([128, 512], f32, tag="y", name="psl",
                               space="PSUM")
                for k in range(KT):
                    nc.tensor.matmul(
                        psl[:pt, 0:16],
                        lhsT=xgb_sb[:, k * C_ + ct * 128:
                                    k * C_ + ct * 128 + pt],
                        rhs=rwe_sb[:, k * 16:(k + 1) * 16],
                        start=(k == 0), stop=(k == KT - 1))
                nc.scalar.activation(gates_sb[:pt, ct:ct + 1],
                                     psl[:pt, 0:1], AF.Copy)

            _PHASE_MARKS.append(('B', nc.next_id()))
            # ---- C: y = gate * (ht.T @ w2t), scatter (f32) to sendbuf ----
            for ct in range(CT_):
                pt = PT[ct]
                ysb = work.tile([128, H], f32, tag="ysb", name="ysb", bufs=3)
                for hh in range(2):
                    psy = psY.tile([128, 512], f32, tag="y", name="psy",
                                   space="PSUM")
                    for i in range(IT):
                        nc.tensor.matmul(
                            psy[:pt, :],
                            lhsT=ht_sb[:, i * C_ + ct * 128:
                                       i * C_ + ct * 128 + pt],
                            rhs=w2_sb[:, i * H + hh * 512:
                                      i * H + hh * 512 + 512],
                            start=(i == 0), stop=(i == IT - 1))
                    nc.scalar.activation(
                        ysb[:pt, hh * 512:(hh + 1) * 512],
                        psy[:pt, :], AF.Copy,
                        scale=gates_sb[:pt, ct:ct + 1])
                nc.sync.dma_start(ybuf[ct * 128:ct * 128 + pt, :],
                                  ysb[:pt, :])

            # permute compact rows into A2A slot order (gathers, not
            # scatters: SWDGE scatter descriptor processing is ~14x
            # more expensive per tile)
            for st in range(SGT):
                gt = work.tile([128, H], f32, tag="gt", name="gt", bufs=4)
                nc.gpsimd.indirect_dma_start(
                    out=gt[:], out_offset=None,
                    in_=ybuf[:, :],
                    in_offset=bass.IndirectOffsetOnAxis(
                        ap=gsnd_sb[:, st:st + 1], axis=0))
                nc.gpsimd.dma_start(sendbuf[st * 128:(st + 1) * 128, :],
                                    gt[:])

            _PHASE_MARKS.append(('C', nc.next_id()))
            # ---- A2A: exchange expert rows to owner cores ----
            nc.gpsimd.collective_compute(
                "AllToAll",
                mybir.AluOpType.bypass,
                replica_groups=[list(range(NCORES))],
                ins=[sendbuf[:, :]],
                outs=[recvbuf[:, :]],
            )

            _PHASE_MARKS.append(('A2A', nc.next_id()))
            # ---- S: shared expert on own 256 tokens (hides the A2A) ----
            hts_sb = sres.tile([128, IT * TSL], bf16, tag="hts_sb",
                               name="hts_sb")
            for i in range(IT):
                psa = psA.tile([128, TSL], f32, tag="a", name="psa_s",
                               space="PSUM")
                psb = psB.tile([128, TSL], f32, tag="b", name="psb_s",
                               space="PSUM")
                for k in range(KT):
                    nc.tensor.matmul(
                        psa[:],
                        lhsT=s1_sb[:, (i * KT + k) * 128:
                                   (i * KT + k + 1) * 128],
                        rhs=xsh_sb[:, k * TSL:(k + 1) * TSL],
                        start=(k == 0), stop=(k == KT - 1))
                for k in range(KT):
                    nc.tensor.matmul(
                        psb[:],
                        lhsT=s3_sb[:, (i * KT + k) * 128:
                                   (i * KT + k + 1) * 128],
                        rhs=xsh_sb[:, k * TSL:(k + 1) * TSL],
                        start=(k == 0), stop=(k == KT - 1))
                sact = work.tile([128, TSL], f32, tag="sact_s", name="sact_s")
                nc.scalar.activation(sact[:], psa[:], AF.Silu)
                nc.vector.tensor_tensor(
                    out=hts_sb[:, i * TSL:(i + 1) * TSL],
                    in0=sact[:], in1=psb[:], op=MUL)

            _PHASE_MARKS.append(('sharedup', nc.next_id()))
            # shared down-proj + gather-add tail per own token tile
            for tt in range(TT):
                ysh = work.tile([128, H], f32, tag="ysh", name="ysh")
                for hh in range(2):
                    psy = psY.tile([128, 512], f32, tag="y", name="psy_s",
                                   space="PSUM")
                    for i in range(IT):
                        nc.tensor.matmul(
                            psy[:],
                            lhsT=hts_sb[:, i * TSL + tt * 128:
                                        i * TSL + tt * 128 + 128],
                            rhs=s2_sb[:, i * H + hh * 512:
                                      i * H + hh * 512 + 512],
                            start=(i == 0), stop=(i == IT - 1))
                    nc.scalar.activation(ysh[:, hh * 512:(hh + 1) * 512],
                                         psy[:], AF.Copy)
                # accumulate the two expert contributions (Pool queue only)
                nc.gpsimd.indirect_dma_start(
                    out=ysh[:], out_offset=None,
                    in_=recvbuf[:, :],
                    in_offset=bass.IndirectOffsetOnAxis(
                        ap=g1i_sb[:, tt:tt + 1], axis=0),
                    compute_op=ADD)
                nc.gpsimd.indirect_dma_start(
                    out=ysh[:], out_offset=None,
                    in_=recvbuf[:, :],
                    in_offset=bass.IndirectOffsetOnAxis(
                        ap=g2i_sb[:, tt:tt + 1], axis=0),
                    compute_op=ADD)
                nc.sync.dma_start(out[tt * 128:(tt + 1) * 128, :], ysh[:])

            _PHASE_MARKS.append(('end', nc.next_id()))
    nc.finalize()
    return nc


def _get_nc(reps=1, cap=None, pcap=None):
    key = (reps, cap or C, pcap or P)
    if key not in _BUILD_CACHE:
        _BUILD_CACHE[key] = _build(reps, cap=cap, pcap=pcap)
    return _BUILD_CACHE[key]


def _route(x2, router_w):
    logits = x2 @ router_w.T                      # [T, E] fp32, dispatch only
    order = np.argsort(-logits, axis=1, kind="stable")[:, :TOPK]
    return order


def _dispatch(order, cap, pcap):
    """Host-side sharding decision: compact lists + gather tables."""
    Tn = order.shape[0]
    rows_e, gsnd_e = [], []
    slot = np.zeros((E, Tn), dtype=np.int64)      # slot of token in expert e
    for e in range(E):
        rows = np.where((order == e).any(axis=1))[0]      # ascending
        ce = len(rows)
        assert ce <= cap, f"expert {e} overflow: {ce} > {cap}"
        owners = rows // TSL
        cnt = np.zeros(E, dtype=np.int64)
        gs = np.zeros(E * pcap, dtype=np.int32)   # slot -> compact row (0 pad)
        for c in range(ce):
            d = owners[c]
            assert cnt[d] < pcap, f"pair ({e},{d}) overflow"
            gs[d * pcap + cnt[d]] = c
            slot[e, rows[c]] = cnt[d]
            cnt[d] += 1
        rows_e.append(rows)
        gsnd_e.append(gs)
    g1, g2 = [], []
    for d in range(E):
        tok = np.arange(d * TSL, (d + 1) * TSL)
        e1, e2 = order[tok, 0], order[tok, 1]
        g1.append((e1 * pcap + slot[e1, tok]).astype(np.int32))
        g2.append((e2 * pcap + slot[e2, tok]).astype(np.int32))
    return rows_e, gsnd_e, g1, g2


def _make_in_maps(x2, router_w, w1, w2, w3, sw1, sw2, sw3,
                  cap=None, pcap=None):
    import ml_dtypes
    bf = ml_dtypes.bfloat16
    cap = cap or C
    pcap = pcap or P
    order = _route(x2, router_w)
    rows_e, gsnd_e, g1, g2 = _dispatch(order, cap, pcap)

    def upw(a):      # [I, H] -> [128, IT, KT*128]: [p,i,k*128+m]=A[i*128+m,k*128+p]
        return np.ascontiguousarray(
            np.asarray(a, np.float32).reshape(IT, 128, KT, 128)
            .transpose(3, 0, 2, 1).reshape(128, IT, KT * 128).astype(bf))

    def dww(a):      # [H, I] -> [128, IT*H]: [p,i*H+h]=A.T[i*128+p,h]
        return np.ascontiguousarray(
            np.asarray(a, np.float32).T.reshape(IT, 128, H)
            .transpose(1, 0, 2).reshape(128, IT * H).astype(bf))

    def xarr(xrows):  # [N, H] -> [128, KT*N]: [p,k*N+c]=x[c,k*128+p]
        n = xrows.shape[0]
        return np.ascontiguousarray(
            xrows.reshape(n, KT, 128).transpose(2, 1, 0)
            .reshape(128, KT * n).astype(bf))

    s1h = upw(sw1)
    s3h = upw(sw3)
    s2h = dww(sw2)
    in_maps = []
    for e in range(E):
        rows = rows_e[e]
        xg_full = np.zeros((cap, H), dtype=np.float32)
        xg_full[:len(rows)] = x2[rows]
        in_maps.append({
            "xgb": xarr(xg_full),
            "xsh": xarr(x2[e * TSL:(e + 1) * TSL]),
            "w1t": upw(w1[e]),
            "w3t": upw(w3[e]),
            "w2t": dww(w2[e]),
            "s1t": s1h,
            "s3t": s3h,
            "s2t": s2h,
            "rwe": np.ascontiguousarray(
                np.tile(np.asarray(router_w[e], np.float32)
                        .reshape(KT, 128, 1), (1, 1, 16))
                .transpose(1, 0, 2).reshape(128, KT * 16).astype(bf)),
            "gsnd": np.ascontiguousarray(
                gsnd_e[e].reshape((E * pcap) // 128, 128).T),
            "g1i": np.ascontiguousarray(g1[e].reshape(TSL // 128, 128).T),
            "g2i": np.ascontiguousarray(g2[e].reshape(TSL // 128, 128).T),
        })
    return in_maps


def kernel(x, router_w, w1, w2, w3, sw1, sw2, sw3):
    from concourse.bass_utils import run_bass_kernel_spmd

    in_dtype = x.dtype
    x2 = np.ascontiguousarray(x.reshape(T, H), dtype=np.float32)
    router_w = np.asarray(router_w, dtype=np.float32)
    order = _route(x2, router_w)
    cmax = max(int((order == e).any(axis=1).sum()) for e in range(E))
    pmax = 0
    for e in range(E):
        rows = np.where((order == e).any(axis=1))[0]
        if len(rows):
            pmax = max(pmax, int(np.bincount(rows // TSL, minlength=E).max()))
    cap = C if cmax <= C else -((-cmax) // 128) * 128
    pcap = P if pmax <= P else -((-pmax) // 32) * 32
    nc = _get_nc(1, cap=cap, pcap=pcap)

    in_maps = _make_in_maps(x2, router_w, w1, w2, w3, sw1, sw2, sw3,
                            cap, pcap)
    res = run_bass_kernel_spmd(nc, in_maps, list(range(NCORES)))
    out = np.concatenate([res.results[i]["out"] for i in range(NCORES)],
                         axis=0)
    return out.reshape(x.shape).astype(in_dtype)
